# revision 1
# baseline (speedup 1.0000x reference)
"""LongT5 transient-global attention on 8 Trainium2 cores — v2.

Sharding: core c = (batch b = c//4, sequence quarter qtr = c%4); each core
computes its 1024 query tokens with 1-block zero-padded K/V halo; the 256
global summary tokens are computed redundantly per core.

v2 vs v1: host-pretransposed hidden (no PE transposes for hiddenT), fp16
Q/K with per-block attention (no wasted score columns), batched DMAs,
per-block PV accumulation with shared denominator column, V-tail and
out-projection matmuls interleaved into the Act-bound attention phase.
"""
import sys, math
sys.path.insert(0, "/opt/trn_rl_repo")
import numpy as np
import ml_dtypes

import concourse.bass as bass
import concourse.mybir as mybir
import concourse.tile as tile
from concourse import bacc
from concourse.masks import make_identity
from concourse.bass_utils import run_bass_kernel_spmd

F32 = mybir.dt.float32
F16 = mybir.dt.float16
BF16 = mybir.dt.bfloat16

B, S, D = 2, 4096, 1024
H, DKV = 16, 64
L = 128                  # block len
G = 256                  # global tokens per batch
GBLK = 16                # tokens per global block
NUM_BUCKETS, MAX_DIST = 32, 128
EPS = 1e-6

TOK_Q = 1024             # query tokens per core
TOK_K = TOK_Q + 2 * L    # halo'd K/V tokens per core
NB = TOK_Q // L          # 8 query blocks per core
GB_CORE = TOK_Q // GBLK  # 64 query-global-block ids per core
WT_W = 512               # local bias table width per head


MM_LABELS = {}
MM_CTX = {"label": "?"}


def _mm(nc, out, lhsT, rhs, start, stop):
    ins = nc.tensor.matmul(out, lhsT, rhs, start=start, stop=stop)
    try:
        MM_LABELS[ins.name] = MM_CTX["label"]
    except Exception:
        pass


def _emit_gsum_group(nc, g, gs_in, t_b16, gs_ps, gsumT):
    """One group of 4 hid_full tiles -> gsumT[:, dc*256 + (g*4+ti)*8 : +8]."""
    for ti in range(4):
        for dc in range(8):
            _mm(nc, gs_ps[:, ti * 64 + dc * 8: ti * 64 + dc * 8 + 8],
                gs_in[:, ti * D + dc * L:(ti * D) + (dc + 1) * L], t_b16,
                True, True)
    # one Act copy, strided into gsumT
    dst = bass.AP(tensor=gsumT.tensor, offset=gsumT.offset + g * 32,
                  ap=[[gsumT.ap[0][0], L], [G, 8], [8, 4], [1, 8]])
    src = bass.AP(tensor=gs_ps.tensor, offset=gs_ps.offset,
                  ap=[[gs_ps.ap[0][0], L], [8, 8], [64, 4], [1, 8]])
    nc.scalar.copy(out=dst, in_=src)


def _emit_gsum_sq(nc, gsumT, pgf):
    sq = pgf.tile([L, 8 * G], F16, tag="sq")
    nc.vector.tensor_mul(out=sq, in0=gsumT, in1=gsumT)
    ones1 = pgf.tile([L, 1], F16, tag="ones1")
    nc.vector.memset(ones1, 1.0)
    return sq, ones1


def _emit_gsum_finish(nc, tc, sq, ones1, gsumT, t_lnw, gnT, pgf, psmall):
    ssum = psmall.tile([L, 512], F32, tag="ssum")
    for dc in range(8):
        _mm(nc, ssum[0:1, 0:G], ones1, sq[:, dc * G:(dc + 1) * G],
            dc == 0, dc == 7)
    eps_t = pgf.tile([1, 1], F32, tag="eps")
    nc.vector.memset(eps_t, EPS)
    sd = pgf.tile([1, G], F32, tag="sd")
    nc.scalar.activation(out=sd, in_=ssum[0:1, 0:G],
                         func=mybir.ActivationFunctionType.Sqrt,
                         bias=eps_t, scale=1.0 / D)
    rstd = pgf.tile([1, G], F32, tag="rstd")
    nc.vector.reciprocal(out=rstd, in_=sd)
    ones_row = pgf.tile([1, L], F32, tag="onesrow")
    nc.vector.memset(ones_row, 1.0)
    rstd_b = psmall.tile([L, 512], F32, tag="ssum", name="rstd_b")
    _mm(nc, rstd_b[:, 0:G], ones_row, rstd, True, True)
    for dc in range(8):
        nc.vector.scalar_tensor_tensor(
            out=gnT[:, dc * G:(dc + 1) * G],
            in0=gsumT[:, dc * G:(dc + 1) * G],
            scalar=t_lnw[:, dc:dc + 1],
            in1=rstd_b[:, 0:G],
            op0=mybir.AluOpType.mult, op1=mybir.AluOpType.mult)


def _build_nc():
    nc = bacc.Bacc(None, target_bir_lowering=False, debug=False)

    hid_kT = nc.declare_dram_parameter("hid_kT", [D, TOK_K], F16, isOutput=False)
    hid_full = nc.declare_dram_parameter("hid_full", [S, D], F16, isOutput=False)
    wq = nc.declare_dram_parameter("wq", [D, D], F16, isOutput=False)
    wk = nc.declare_dram_parameter("wk", [D, D], F16, isOutput=False)
    wv = nc.declare_dram_parameter("wv", [D, D], F16, isOutput=False)
    wo = nc.declare_dram_parameter("wo", [D, D], F16, isOutput=False)
    b16 = nc.declare_dram_parameter("b16", [L, 8], F16, isOutput=False)
    wtab = nc.declare_dram_parameter("wtab", [L, H * WT_W], F16, isOutput=False)
    sideb = nc.declare_dram_parameter("sideb", [L, 2 * H * GB_CORE], F16, isOutput=False)
    lnw = nc.declare_dram_parameter("lnw", [L, 8], F32, isOutput=False)
    outT = nc.declare_dram_parameter("outT", [D, TOK_Q], F16, isOutput=True)

    def load_weight(dst, w, fgs=(0, 1)):
        # dst[p, (fg*8+dc)*512 + j] = w[dc*128+p, fg*512+j]
        wb = w[:, :]
        for fg in fgs:
            d = bass.AP(tensor=dst.tensor, offset=dst.offset + fg * 8 * 512,
                        ap=[[dst.ap[0][0], L], [512, 8], [1, 512]])
            s = bass.AP(tensor=wb.tensor, offset=wb.offset + fg * 512,
                        ap=[[D, L], [L * D, 8], [1, 512]])
            nc.sync.dma_start(out=d, in_=s)

    def wslice(wsb, fg, dc, j0, wdt):
        c0 = (fg * 8 + dc) * 512 + j0
        return wsb[:, c0:c0 + wdt]

    with tile.TileContext(nc) as tc:
        with tc.tile_pool(name="persist", bufs=1) as pp, \
             tc.tile_pool(name="acts", bufs=1) as pa:
            t_b16 = pp.tile([L, 8], F16)
            t_lnw = pp.tile([L, 8], F32)
            ident = pp.tile([L, L], F16)
            t_wtab = pp.tile([L, H * WT_W], F16)
            t_sideb = pp.tile([L, 2 * H * GB_CORE], F16)

            hiddenT = pa.tile([L, 8 * TOK_K], F16)   # [din-part, dc x tok]
            QT = pa.tile([L, 8 * TOK_Q], F16)        # [fc-feat-part, fc x tok]
            KT = pa.tile([L, 8 * TOK_K], F16)
            sideKT = pa.tile([L, 8 * G], F16)
            V_aug = [pa.tile([L, H * (DKV + 1)], BF16, tag=f"vaug{t}", name=f"vaug{t}")
                     for t in range(10)]
            sideV_aug = [pa.tile([L, H * (DKV + 1)], BF16, tag=f"svaug{t}", name=f"svaug{t}")
                         for t in range(2)]
            gsumT = pa.tile([L, 8 * G], F32)         # [din-part, dc x g]
            gnT = pa.tile([L, 8 * G], F16)
            attnT = pa.tile([L, 8 * TOK_Q], F16)     # [fc-feat-part, fc x tok]

            make_identity(nc, ident)
            # warm up the PE p-state ramp before the first weight DMA lands
            with tc.tile_pool(name="pwarm", bufs=1, space="PSUM") as pwarm:
                wps = pwarm.tile([L, L], F16)
                for _ in range(12):
                    nc.tensor.transpose(wps, ident, ident)
            for t in range(10):
                nc.gpsimd.memset(V_aug[t], 1.0)
            for t in range(2):
                nc.gpsimd.memset(sideV_aug[t], 1.0)

            # ---------------- phase 1: QT + gsum ----------------
            with tc.tile_pool(name="pwq", bufs=1) as pwq, \
                 tc.tile_pool(name="pwk", bufs=1) as pwk, \
                 tc.tile_pool(name="pgin", bufs=2) as pgin, \
                 tc.tile_pool(name="pgf", bufs=1) as pgf:
                wq_sb = pwq.tile([L, 16 * 512], F16)
                wk_sb = pwk.tile([L, 16 * 512], F16)
                hkb = hid_kT[:, :]
                wqb = wq[:, :]

                def hT_dma(t0, t1, c0, c1):
                    d = bass.AP(tensor=hiddenT.tensor,
                                offset=hiddenT.offset + c0 * TOK_K + t0,
                                ap=[[hiddenT.ap[0][0], L], [TOK_K, c1 - c0], [1, t1 - t0]])
                    s = bass.AP(tensor=hkb.tensor,
                                offset=hkb.offset + c0 * L * TOK_K + t0,
                                ap=[[TOK_K, L], [L * TOK_K, c1 - c0], [1, t1 - t0]])
                    nc.sync.dma_start(out=d, in_=s)

                def wq_dma(c0, c1):
                    d = bass.AP(tensor=wq_sb.tensor, offset=wq_sb.offset + c0 * 512,
                                ap=[[wq_sb.ap[0][0], L], [512, c1 - c0], [1, 512]])
                    s = bass.AP(tensor=wqb.tensor, offset=wqb.offset + c0 * L * D,
                                ap=[[D, L], [L * D, c1 - c0], [1, 512]])
                    nc.sync.dma_start(out=d, in_=s)

                wq_dma(0, 4)
                hT_dma(0, 640, 0, 4)
                wq_dma(4, 8)
                hT_dma(0, 640, 4, 8)
                load_weight(wq_sb, wq, fgs=(1,))
                hT_dma(640, TOK_K, 0, 8)
                load_weight(wk_sb, wk)
                nc.sync.dma_start(out=t_b16, in_=b16[:])
                nc.sync.dma_start(out=t_lnw, in_=lnw[:])

                gs_in = []
                for g in range(8):
                    gt = pgin.tile([L, 4 * D], F16, tag="gsin", name=f"gsin{g}")
                    d = bass.AP(tensor=gt.tensor, offset=gt.offset,
                                ap=[[gt.ap[0][0], L], [D, 4], [1, D]])
                    hfb = hid_full[:, :]
                    s = bass.AP(tensor=hfb.tensor,
                                offset=hfb.offset + g * 4 * L * D,
                                ap=[[D, L], [L * D, 4], [1, D]])
                    nc.sync.dma_start(out=d, in_=s)
                    gs_in.append(gt)
                nc.sync.dma_start(out=t_wtab, in_=wtab[:])
                nc.sync.dma_start(out=t_sideb, in_=sideb[:])

                # QT, th-outer so th=0 only needs the first hiddenT DMA.
                # First four groups run dc0-3 across all four fc before dc4-7
                # so the PE has work while the second dc-half DMAs land.
                ppjA = tc.alloc_tile_pool(name="ppjA", bufs=4, space="PSUM")
                if True:
                    def qt_mm(fc, th, pq, dcs):
                        fg, fl = fc // 4, fc % 4
                        for dc in dcs:
                            _mm(nc, pq, wslice(wq_sb, fg, dc, fl * L, L),
                                hiddenT[:, dc * TOK_K + L + th * 512:
                                        dc * TOK_K + L + (th + 1) * 512],
                                dc == 0, dc == 7)

                    def qt_store(fc, th, pq):
                        nc.vector.tensor_copy(
                            out=QT[:, fc * TOK_Q + th * 512:
                                   fc * TOK_Q + (th + 1) * 512], in_=pq)

                    pqs = [ppjA.tile([L, 512], F32, tag="ppj", name=f"pq{fc}_0")
                           for fc in range(4)]
                    for fc in range(4):
                        qt_mm(fc, 0, pqs[fc], range(4))
                    for fc in range(4):
                        qt_mm(fc, 0, pqs[fc], range(4, 8))
                        qt_store(fc, 0, pqs[fc])
                    for th in range(2):
                        for fc in (range(4, 8) if th == 0 else range(8)):
                            pq = ppjA.tile([L, 512], F32, tag="ppj",
                                           name=f"pq{fc}_{th}")
                            qt_mm(fc, th, pq, range(8))
                            qt_store(fc, th, pq)

                # ---------------- phase 2: KT, gsum interleaved ----------------
                ppjB = ppjA
                with tc.tile_pool(name="pgs", bufs=2, space="PSUM") as pgs:
                    for fg in range(2):
                        for fl in range(4):
                            fc = fg * 4 + fl
                            for th in range(3):
                                w_ = 512 if th < 2 else 256
                                pk = ppjB.tile([L, 512], F32, tag="ppj",
                                               name=f"pk{fc}_{th}")
                                for dc in range(8):
                                    _mm(nc, pk[:, :w_],
                                        wslice(wk_sb, fg, dc, fl * L, L),
                                        hiddenT[:, dc * TOK_K + th * 512:
                                                dc * TOK_K + th * 512 + w_],
                                        dc == 0, dc == 7)
                                nc.vector.tensor_copy(
                                    out=KT[:, fc * TOK_K + th * 512:
                                           fc * TOK_K + th * 512 + w_],
                                    in_=pk[:, :w_])
                            if fc < 4:
                                for g in (2 * fc, 2 * fc + 1):
                                    gs_ps = pgs.tile([L, 256], F32, tag="gsps",
                                                     name=f"gsps{g}")
                                    _emit_gsum_group(nc, g, gs_in[g], t_b16,
                                                     gs_ps, gsumT)
                            if fc == 3:
                                sq, ones1 = _emit_gsum_sq(nc, gsumT, pgf)
                            if fc == 5:
                                with tc.tile_pool(name="psmall", bufs=2,
                                                  space="PSUM") as psmall:
                                    _emit_gsum_finish(nc, tc, sq, ones1, gsumT,
                                                      t_lnw, gnT, pgf, psmall)
                    # sideKT (needs gnT from the finish chain just above)
                    for fg in range(2):
                        for fl in range(4):
                            fc = fg * 4 + fl
                            psk = ppjB.tile([L, 512], F32, tag="ppj",
                                            name=f"psk{fc}")
                            for dc in range(8):
                                _mm(nc, psk[:, :G],
                                    wslice(wk_sb, fg, dc, fl * L, L),
                                    gnT[:, dc * G:(dc + 1) * G],
                                    dc == 0, dc == 7)
                            nc.vector.tensor_copy(
                                out=sideKT[:, fc * G:(fc + 1) * G],
                                in_=psk[:, :G])

                ppjA.release()

            # ---------------- phase 3: V head + attn + outproj ----------------
            with tc.tile_pool(name="pwv", bufs=1) as pwv, \
                 tc.tile_pool(name="pwo", bufs=1) as pwo:
                wv_sb = pwv.tile([L, 16 * 512], F16)
                wo_sb = pwo.tile([L, 16 * 512], F16)
                load_weight(wv_sb, wv)
                load_weight(wo_sb, wo)

                def v_matmul(t, fh, dc, pv_ps):
                    _mm(nc, pv_ps,
                        hiddenT[:, dc * TOK_K + t * L: dc * TOK_K + (t + 1) * L],
                        wv_sb[:, (fh * 8 + dc) * 512:(fh * 8 + dc + 1) * 512],
                        dc == 0, dc == 7)

                def v_store(t, fh, pv_ps):
                    dst = bass.AP(tensor=V_aug[t].tensor,
                                  offset=V_aug[t].offset + fh * 8 * (DKV + 1),
                                  ap=[[V_aug[t].ap[0][0], L], [DKV + 1, 8], [1, DKV]])
                    nc.vector.tensor_copy(
                        out=dst, in_=pv_ps.rearrange("p (h d) -> p h d", h=8))

                with tc.tile_pool(name="pst", bufs=2, space="PSUM") as pst, \
                     tc.tile_pool(name="ppv", bufs=2, space="PSUM") as ppv, \
                     tc.tile_pool(name="pet", bufs=11) as pet, \
                     tc.tile_pool(name="pat", bufs=2) as pat, \
                     tc.tile_pool(name="psc", bufs=4) as psc, \
                     tc.tile_pool(name="pot", bufs=3) as pot:

                    wt_ps = t_wtab.ap[0][0]
                    sb_ps = t_sideb.ap[0][0]

                    # V preamble: tiles 0..3 + sideV
                    ppjC = tc.alloc_tile_pool(name="ppjC", bufs=2, space="PSUM")
                    if True:
                        for t in range(3):
                            for fh in range(2):
                                pv_ps = ppjC.tile([L, 512], F32, tag="ppj",
                                                  name=f"pvp{t}_{fh}")
                                for dc in range(8):
                                    v_matmul(t, fh, dc, pv_ps)
                                v_store(t, fh, pv_ps)
                        for gc in range(2):
                            for fh in range(2):
                                pv_ps = ppjC.tile([L, 512], F32, tag="ppj",
                                                  name=f"psv{gc}_{fh}")
                                for dc in range(8):
                                    _mm(nc, pv_ps,
                                        gnT[:, dc * G + gc * L: dc * G + (gc + 1) * L],
                                        wv_sb[:, (fh * 8 + dc) * 512:
                                              (fh * 8 + dc + 1) * 512],
                                        dc == 0, dc == 7)
                                dst = bass.AP(
                                    tensor=sideV_aug[gc].tensor,
                                    offset=sideV_aug[gc].offset + fh * 8 * (DKV + 1),
                                    ap=[[sideV_aug[gc].ap[0][0], L], [DKV + 1, 8], [1, DKV]])
                                nc.vector.tensor_copy(
                                    out=dst,
                                    in_=pv_ps.rearrange("p (h d) -> p h d", h=8))

                        # ---- attention blocks with deferred PE matmuls
                        # (V-tail / out-proj) spread across the 16 heads ----
                        def attn_block(b, fills):
                            # Software-pipelined by one head: PV(h-1) is
                            # emitted after scores(h)+fills so its et-mul
                            # deps are satisfied by the time PE reaches it
                            # (the PE wait-queue is only 4 deep).
                            fi = 0
                            nit = H + 5
                            per_head = (len(fills) + nit - 1) // nit if fills else 0
                            pv_tiles = []
                            ets = {}

                            def emit_pv(h, fill1=None):
                                if h % 4 == 0:
                                    pv = ppv.tile([L, 4 * (DKV + 1)], F32,
                                                  tag="pv", name=f"pv{b}_{h // 4}")
                                    pv_tiles.append(pv)
                                pv = pv_tiles[h // 4]
                                et = ets.pop(h)
                                MM_CTX["label"] = f"pv b{b} h{h}"
                                for c in range(5):
                                    if c == 3 and fill1 is not None:
                                        fill1()
                                    if c < 3:
                                        rhs = V_aug[b + c][:, h * (DKV + 1):
                                                           (h + 1) * (DKV + 1)]
                                    else:
                                        rhs = sideV_aug[c - 3][:, h * (DKV + 1):
                                                               (h + 1) * (DKV + 1)]
                                    _mm(nc, pv[:, (h % 4) * (DKV + 1):
                                               (h % 4 + 1) * (DKV + 1)],
                                        et[:, c * L:(c + 1) * L], rhs,
                                        c == 0, c == 4)

                            def emit_norm(h):
                                g2 = h // 4
                                pv = pv_tiles[g2]
                                rec = psc.tile([L, 4], F32, tag="rec",
                                               name=f"rec{b}_{g2}")
                                den = bass.AP(tensor=pv.tensor,
                                              offset=pv.offset + DKV,
                                              ap=[[pv.ap[0][0], L], [DKV + 1, 4]])
                                nc.vector.reciprocal(out=rec, in_=den)
                                if g2 == 0:
                                    asb = pat.tile([L, H * DKV], F16,
                                                   tag="asb", name=f"asb{b}")
                                    attn_block.asb = asb
                                asb = attn_block.asb
                                pvv = bass.AP(tensor=pv.tensor, offset=pv.offset,
                                              ap=[[pv.ap[0][0], L], [DKV + 1, 4], [1, DKV]])
                                recb = bass.AP(tensor=rec.tensor, offset=rec.offset,
                                               ap=[[rec.ap[0][0], L], [1, 4], [0, DKV]])
                                dst = bass.AP(tensor=asb.tensor,
                                              offset=asb.offset + g2 * 4 * DKV,
                                              ap=[[asb.ap[0][0], L], [DKV, 4], [1, DKV]])
                                nc.vector.tensor_mul(out=dst, in0=pvv, in1=recb)
                                if g2 % 2 == 1:
                                    half = g2 // 2

                                    def do_tr(asb=asb, half=half, b=b):
                                        ptr = ppv.tile([L, 512], F16, tag="pv",
                                                       name=f"ptr{b}_{half}")
                                        for j in range(4):
                                            icc = half * 4 + j
                                            nc.tensor.transpose(
                                                ptr[:, j * L:(j + 1) * L],
                                                asb[:, icc * L:(icc + 1) * L], ident)
                                        dst2 = bass.AP(
                                            tensor=attnT.tensor,
                                            offset=attnT.offset + (half * 4) * TOK_Q + b * L,
                                            ap=[[attnT.ap[0][0], L], [TOK_Q, 4], [1, L]])
                                        nc.vector.tensor_copy(
                                            out=dst2,
                                            in_=ptr.rearrange("p (c t) -> p c t", c=4))
                                    if half == 0:
                                        attn_block.pending_tr0 = do_tr
                                    else:
                                        attn_block.pending_tr = do_tr

                            for it in range(nit):
                                def fill1():
                                    nonlocal fi
                                    if fi < len(fills) and fi < (it + 1) * per_head:
                                        fills[fi]()
                                        fi += 1
                                if it < H:
                                    h = it
                                    fc, p0 = h // 2, (h % 2) * DKV
                                    st = pst.tile([L, 640], F32, tag="st",
                                                  name=f"st{b}_{h}")
                                    qtap = QT[p0:p0 + DKV,
                                              fc * TOK_Q + b * L: fc * TOK_Q + (b + 1) * L]
                                    MM_CTX["label"] = f"score_loc b{b} h{h}"
                                    for c in range(3):
                                        _mm(nc, st[:, c * L:(c + 1) * L],
                                            KT[p0:p0 + DKV,
                                               fc * TOK_K + (b + c) * L:
                                               fc * TOK_K + (b + c + 1) * L],
                                            qtap, True, True)
                                    MM_CTX["label"] = f"score_glob b{b} h{h}"
                                    for gc in range(2):
                                        _mm(nc, st[:, (3 + gc) * L:(4 + gc) * L],
                                            sideKT[p0:p0 + DKV,
                                                   fc * G + gc * L: fc * G + (gc + 1) * L],
                                            qtap, True, True)
                                    et = pet.tile([L, 640], BF16, tag="et",
                                                  name=f"et{b}_{h}")
                                    ets[h] = et
                                    nc.scalar.activation(
                                        out=et, in_=st,
                                        func=mybir.ActivationFunctionType.Exp)
                                    loc = bass.AP(
                                        tensor=t_wtab.tensor,
                                        offset=t_wtab.offset + h * WT_W + 127,
                                        ap=[[wt_ps, L], [L, 3], [-1, L]])
                                    nc.vector.tensor_mul(
                                        out=et[:, 0:384].rearrange("p (c q) -> p c q", c=3),
                                        in0=et[:, 0:384].rearrange("p (c q) -> p c q", c=3),
                                        in1=loc)
                                    sid = bass.AP(
                                        tensor=t_sideb.tensor,
                                        offset=t_sideb.offset + h * GB_CORE + b * 8,
                                        ap=[[sb_ps, L], [H * GB_CORE, 2], [1, 8], [0, 16]])
                                    nc.gpsimd.tensor_mul(
                                        out=et[:, 384:640].rearrange(
                                            "p (c b r) -> p c b r", c=2, b=8),
                                        in0=et[:, 384:640].rearrange(
                                            "p (c b r) -> p c b r", c=2, b=8),
                                        in1=sid)
                                if it == 4 and attn_block.pending_tr is not None:
                                    attn_block.pending_tr()
                                    attn_block.pending_tr = None
                                if it == 17 and attn_block.pending_tr0 is not None:
                                    attn_block.pending_tr0()
                                    attn_block.pending_tr0 = None
                                if it >= 5:
                                    emit_pv(it - 5)
                                    if (it - 5) % 4 == 3:
                                        emit_norm(it - 5)
                                while fi < len(fills) and fi < (it + 1) * per_head:
                                    fills[fi]()
                                    fi += 1
                            while fi < len(fills):
                                fills[fi]()
                                fi += 1

                        vps_holder = {}

                        def v_fill(t, fh, dc, pool=None):
                            def f():
                                if dc == 0:
                                    vps_holder[(t, fh)] = (pool or ppjC).tile(
                                        [L, 512], F32,
                                        tag="ppj",
                                        name=f"pvp{t}_{fh}")
                                MM_CTX["label"] = f"vfill t{t} fh{fh} dc{dc}"
                                v_matmul(t, fh, dc, vps_holder[(t, fh)])
                                if dc == 7:
                                    v_store(t, fh, vps_holder.pop((t, fh)))
                            return f

                        def v_fills(t, pool=None):
                            return [v_fill(t, fh, dc, pool)
                                    for fh in range(2) for dc in range(8)]

                        attn_block.pending_tr = None
                        attn_block.pending_tr0 = None

                        # blocks 0..2 carry V tiles 3..8 (two tiles each)
                        for b in range(3):
                            fills = v_fills(3 + 2 * b) + v_fills(4 + 2 * b)
                            attn_block(b, fills)

                    # ---- attn blocks 3..7 + outproj interleaved ----
                    ppo = ppjC
                    if True:
                        op_holder = {}
                        ot_hold = {}

                        def op_fill(q, nco, ic):
                            def f():
                                if ic == 0:
                                    op_holder[(q, nco)] = ppo.tile(
                                        [L, 256], F32, tag="ppo",
                                        name=f"po{q}_{nco}")
                                po = op_holder[(q, nco)]
                                MM_CTX["label"] = f"op q{q} nco{nco} ic{ic}"
                                fg, nl = nco // 4, nco % 4
                                _mm(nc, po,
                                    wslice(wo_sb, fg, ic, nl * L, L),
                                    attnT[:, ic * TOK_Q + q * 256:
                                          ic * TOK_Q + (q + 1) * 256],
                                    ic == 0, ic == 7)
                                if ic == 7:
                                    po2 = op_holder.pop((q, nco))
                                    ot = pot.tile([L, 256], F16, tag="ot",
                                                  name=f"ot{q}_{nco}")
                                    nc.vector.tensor_copy(out=ot, in_=po2)
                                    nc.sync.dma_start(
                                        out=outT[nco * L:(nco + 1) * L,
                                                 q * 256:(q + 1) * 256],
                                        in_=ot)
                            return f

                        def ob_fill(blk, nco, ic):
                            # out-proj over one attnT block (128 tokens);
                            # paired blocks share an ot tile and one DMA.
                            def f():
                                key = (blk, nco)
                                pair = blk // 2
                                if ic == 0:
                                    op_holder[key] = ppo.tile(
                                        [L, L], F32, tag="ppj",
                                        name=f"pob{blk}_{nco}")
                                po = op_holder[key]
                                MM_CTX["label"] = f"ob blk{blk} nco{nco} ic{ic}"
                                fg, nl = nco // 4, nco % 4
                                _mm(nc, po,
                                    wslice(wo_sb, fg, ic, nl * L, L),
                                    attnT[:, ic * TOK_Q + blk * L:
                                          ic * TOK_Q + (blk + 1) * L],
                                    ic == 0, ic == 7)
                                if ic == 7:
                                    po2 = op_holder.pop(key)
                                    if blk % 2 == 0:
                                        ot_hold[(pair, nco)] = pot.tile(
                                            [L, 256], F16, tag="otb", bufs=16,
                                            name=f"otb{pair}_{nco}")
                                    ot = ot_hold[(pair, nco)]
                                    nc.vector.tensor_copy(
                                        out=ot[:, (blk % 2) * L:(blk % 2 + 1) * L],
                                        in_=po2)
                                    if blk % 2 == 1:
                                        nc.sync.dma_start(
                                            out=outT[nco * L:(nco + 1) * L,
                                                     pair * 256:(pair + 1) * 256],
                                            in_=ot_hold.pop((pair, nco)))
                            return f

                        def ob_fills(blk, ncos=range(8)):
                            return [ob_fill(blk, nco, ic)
                                    for nco in ncos for ic in range(8)]

                        # out-proj chunk o_b needs attnT block b; water-fill
                        # across blocks 3..7 so the late blocks (which only
                        # unlock one chunk each) still have enough PE work.
                        attn_block(3, v_fills(9, ppo) + ob_fills(0))
                        attn_block(4, ob_fills(1) + ob_fills(2, range(4)))
                        attn_block(5, ob_fills(2, range(4, 8)) + ob_fills(3))
                        attn_block(6, ob_fills(4) + ob_fills(5, range(4)))
                        attn_block(7, ob_fills(5, range(4, 8)) + ob_fills(6))
                        if attn_block.pending_tr0 is not None:
                            attn_block.pending_tr0()
                            attn_block.pending_tr0 = None
                        if attn_block.pending_tr is not None:
                            attn_block.pending_tr()
                            attn_block.pending_tr = None
                        for f in ob_fills(7):                  # block 7 tail
                            f()
                        ppjC.release()

    nc.finalize()
    return nc


# ---------------- host-side table construction ----------------

def _rel_bucket_np(rp):
    """Bit-faithful port of reference _rel_bucket via jax f32 on CPU."""
    import jax
    import jax.numpy as jnp
    with jax.default_device(jax.devices("cpu")[0]):
        rp = jnp.asarray(rp)
        nb = NUM_BUCKETS // 2
        buckets = jnp.where(rp > 0, nb, 0).astype(jnp.int32)
        rpa = jnp.abs(rp)
        max_exact = nb // 2
        is_small = rpa < max_exact
        rp_f = jnp.maximum(rpa, 1).astype(jnp.float32)
        rp_large = max_exact + (jnp.log(rp_f / max_exact) / math.log(MAX_DIST / max_exact)
                                * (nb - max_exact)).astype(jnp.int32)
        rp_large = jnp.minimum(rp_large, nb - 1)
        out = buckets + jnp.where(is_small, rpa.astype(jnp.int32), rp_large)
        return np.asarray(out)


def _make_tables(rel_bias, global_rel_bias, qtr):
    # local: value(p, j) = exp(rel_bias[bucket(j - 255 + p)]) if |d|<128 else 0
    d = np.arange(WT_W)[None, :] - 255 + np.arange(L)[:, None]   # (128, 512)
    dc = np.clip(d, -383, 384)
    buck = _rel_bucket_np(dc.reshape(-1)).reshape(dc.shape)
    wv = np.exp(rel_bias[buck, :].astype(np.float64)).astype(np.float32)  # (128,512,H)
    wv[np.abs(d) >= L] = 0.0
    wtab = np.empty((L, H * WT_W), np.float16)
    for h in range(H):
        wtab[:, h * WT_W:(h + 1) * WT_W] = wv[:, :, h].astype(np.float16)
    # side: sideb[p, gc*H*GB + h*GB + gb] = exp(grel[bucket(g - (qtr*64+gb)), h])
    g = np.arange(G)
    gb_abs = qtr * GB_CORE + np.arange(GB_CORE)
    srel = g[:, None] - gb_abs[None, :]           # (256, 64)
    sbuck = _rel_bucket_np(srel)
    svals = np.exp(global_rel_bias[sbuck, :].astype(np.float64)).astype(np.float32)
    sideb = np.empty((L, 2 * H * GB_CORE), np.float16)
    for gc in range(2):
        for h in range(H):
            sideb[:, gc * H * GB_CORE + h * GB_CORE: gc * H * GB_CORE + (h + 1) * GB_CORE] = \
                svals[gc * L:(gc + 1) * L, :, h].astype(np.float16)
    return wtab, sideb


_NC_CACHE = {}


def kernel(hidden_states, mask, Wq, Wk, Wv, Wo, rel_bias, global_rel_bias, ln_weight):
    hidden_states = np.asarray(hidden_states, np.float32)
    Wq, Wk, Wv, Wo = (np.asarray(w, np.float16) for w in (Wq, Wk, Wv, Wo))
    rel_bias = np.asarray(rel_bias, np.float32)
    global_rel_bias = np.asarray(global_rel_bias, np.float32)
    ln_weight = np.asarray(ln_weight, np.float32)

    if "nc" not in _NC_CACHE:
        _NC_CACHE["nc"] = _build_nc()
    nc = _NC_CACHE["nc"]

    b16 = np.zeros((L, 8), np.float16)
    for t in range(L):
        b16[t, t // GBLK] = 1.0
    lnw = ln_weight.reshape(8, L).T.copy()

    hs16 = hidden_states.astype(np.float16)
    in_maps = []
    for c in range(8):
        b, qtr = c // 4, c % 4
        lo = qtr * TOK_Q - L
        hk = np.zeros((TOK_K, D), np.float16)
        s0, s1 = max(lo, 0), min(lo + TOK_K, S)
        hk[s0 - lo: s1 - lo] = hs16[b, s0:s1]
        wtab, sideb = _make_tables(rel_bias, global_rel_bias, qtr)
        in_maps.append({
            "hid_kT": np.ascontiguousarray(hk.T), "hid_full": hs16[b],
            "wq": Wq, "wk": Wk, "wv": Wv, "wo": Wo,
            "b16": b16, "wtab": wtab, "sideb": sideb, "lnw": lnw,
        })

    res = run_bass_kernel_spmd(nc, in_maps, core_ids=list(range(8)))
    out = np.empty((B, S, D), np.float32)
    for c in range(8):
        b, qtr = c // 4, c % 4
        out[b, qtr * TOK_Q:(qtr + 1) * TOK_Q, :] = \
            res.results[c]["outT"].T.astype(np.float32)
    return out

